# revision 11
# baseline (speedup 1.0000x reference)
"""Trainium2 Bass kernel for MinimalLBS (B=32, T=128, N=2048, J=52, Jb=21, L=16).

Strategy: data-parallel over B across 8 NeuronCores (4 samples per core).

Key algebraic restructure vs the naive path ("Psi-trick"):
  sens[n,i,t] = sum_{k,j} w[n,k] * A_aug[k,i,j,t] * vh[n,j,t]
with vh = vth + dh, vth = (v_template, 1), dh = (delta, 0),
delta = blend_shape + pose_offsets (small, ~0.05 sigma):

  S1[n,i,t] = sum_{(k,j)} Psi[n,(k,j)] * A_aug[(k,j),i,t]   (Psi = w (x) vth,
              t-independent -> host-computed, one bf16 matmul K=209; absorbs
              v_template, translation and the homogeneous j=3 column)
  S2[n,i,t] = sum_{j<3} ts[n,i,j,t] * delta[n,j,t]          (small correction;
              all inputs fp8 DoubleRow matmuls at 0.5 cycles/row)
  sens = S1 + S2

Per chunk of 128 vertices (per sample):
  PE : delta   = pd8^T @ pft8   3x fp8-DR matmuls (K=206: 189 posedirs rows +
                 16 shapedirs rows + pad; v_template NOT included)   -> PSUM sm
       ts9     = wt8^T @ ar8    3x fp8-DR matmuls (K=52, j<3 only)   -> PSUM ts
       S1      = psit^T @ arb   2x bf16 matmuls (K=209)              -> PSUM sm
                 (sm region reused: delta is evacuated before S1 lands)
  ACT: delta evac (scale 1/64 undoes the fp8 range prescale), ts9 evac on even
       chunks (enables 2x-rate DVE multiply)
  DVE/Pool: pm = ts9*delta_b, j-sum, final +S1 -> bf16 out, roles alternating
       by chunk parity to balance engine load.
"""

import sys

sys.path.insert(0, "/opt/trn_rl_repo")

import math

import ml_dtypes
import numpy as np

import concourse.bacc as bacc
import concourse.bass as bass
import concourse.mybir as mybir
import concourse.tile as tile
from concourse import bass_utils, masks

F32 = mybir.dt.float32
BF16 = mybir.dt.bfloat16
F8 = mybir.dt.float8e4
NPBF16 = ml_dtypes.bfloat16
NPF8 = ml_dtypes.float8_e4m3fn
DR = mybir.MatmulPerfMode.DoubleRow

B, T, N, JB, J, L = 32, 128, 2048, 21, 52, 16
NCORES = 8
NB = B // NCORES          # samples per core
PF = JB * 9               # 189 pose-feature dims
NCH = N // 128            # n-chunks per sample
KD = PF + L + 1           # 206 logical K for the delta matmul (pad row last)
KDH = KD // 2             # 103
KS = J                    # 52 logical K for the ts9 matmul
KSH = KS // 2             # 26
KT = J * 4 + 1            # 209 logical K for the S1 matmul
KT0 = 128
KT1 = KT - 128            # 81
PD_SCALE = 64.0           # fp8 range prescale for posedirs/shapedirs

_CACHED = {}


def _build_nc():
    nc = bacc.Bacc("TRN2", target_bir_lowering=False, debug=False)

    pose_d = nc.dram_tensor("pose", [T, NB, JB, 3], F32, kind="ExternalInput")
    pd8_d = nc.dram_tensor("pd8", [NB, 3, KDH, 2, N], F8, kind="ExternalInput")
    beta8_d = nc.dram_tensor("beta8", [NB, L, T], F8, kind="ExternalInput")
    wt8_d = nc.dram_tensor("wt8", [NB, KSH, 2, N], F8, kind="ExternalInput")
    ar8_d = nc.dram_tensor("ar8", [NB, KSH, 2, 3, 3, T], F8, kind="ExternalInput")
    psit_d = nc.dram_tensor("psit", [NB, KT, N], BF16, kind="ExternalInput")
    arb_d = nc.dram_tensor("arb", [NB, KT, 3, T], BF16, kind="ExternalInput")
    out_d = nc.dram_tensor("out", [NB, 128, NCH, 3, T], BF16, kind="ExternalOutput")

    with tile.TileContext(nc) as tc:
        with (
            tc.tile_pool(name="const", bufs=1) as p_const,
            tc.tile_pool(name="rod", bufs=1) as p_rod,
            tc.tile_pool(name="big", bufs=2) as p_big,
            tc.tile_pool(name="small", bufs=2) as p_small,
            tc.tile_pool(name="mv", bufs=10) as p_mv,
            tc.tile_pool(name="sm", bufs=2, space="PSUM") as ps_sm,
            tc.tile_pool(name="ts", bufs=2, space="PSUM") as ps_ts,
        ):
            ident = p_const.tile([128, 128], BF16)
            masks.make_identity(nc, ident[:])
            cst = p_const.tile([128, 2], F32)
            nc.vector.memset(cst[:, 0:1], math.pi / 2)
            nc.vector.memset(cst[:, 1:2], 1.0)

            # ---- Rodrigues for all NB samples at once: pose [t, nb, jb, 3]
            po = p_rod.tile([T, NB, JB, 3], F32)
            nc.sync.dma_start(po[:], pose_d[:])
            sq = p_rod.tile([T, NB, JB, 3], F32)
            nc.vector.tensor_tensor(sq[:], po[:], po[:], mybir.AluOpType.mult)
            a2 = p_rod.tile([T, NB, JB], F32)
            nc.vector.tensor_tensor(
                a2[:], sq[:, :, :, 0], sq[:, :, :, 1], mybir.AluOpType.add
            )
            a2b = p_rod.tile([T, NB, JB], F32)
            nc.vector.tensor_tensor(a2b[:], a2[:], sq[:, :, :, 2], mybir.AluOpType.add)
            a2c = p_rod.tile([T, NB, JB], F32)
            nc.vector.tensor_scalar_max(a2c[:], a2b[:], 1e-16)
            ang = p_rod.tile([T, NB, JB], F32)
            nc.scalar.sqrt(ang[:], a2c[:])
            inv = p_rod.tile([T, NB, JB], F32)
            nc.vector.reciprocal(inv[:], ang[:])
            s = p_rod.tile([T, NB, JB], F32)
            nc.scalar.activation(s[:], ang[:], mybir.ActivationFunctionType.Sin)
            co = p_rod.tile([T, NB, JB], F32)
            nc.scalar.activation(
                co[:], ang[:], mybir.ActivationFunctionType.Sin, bias=cst[:, 0:1]
            )
            u = p_rod.tile([T, NB, JB], F32)
            nc.scalar.activation(
                u[:], co[:], mybir.ActivationFunctionType.Identity,
                bias=cst[:, 1:2], scale=-1.0,
            )
            ax = p_rod.tile([T, NB, JB, 3], F32)
            nc.vector.tensor_tensor(
                ax[:], po[:], inv[:].unsqueeze(3).broadcast_to((T, NB, JB, 3)),
                mybir.AluOpType.mult,
            )

            pf = p_rod.tile([T, NB, JB, 9], BF16)

            def axc(i):
                return ax[:, :, :, i]

            prods = {}
            for (a, b2), nm in [
                ((0, 0), "xx"), ((1, 1), "yy"), ((2, 2), "zz"),
                ((0, 1), "xy"), ((0, 2), "xz"), ((1, 2), "yz"),
            ]:
                t_ = p_rod.tile([T, NB, JB], F32, tag=f"pr_{nm}")
                nc.gpsimd.tensor_tensor(t_[:], axc(a), axc(b2), mybir.AluOpType.mult)
                prods[nm] = t_
            qs = {}
            for i, nm in [(0, "qx"), (1, "qy"), (2, "qz")]:
                t_ = p_rod.tile([T, NB, JB], F32, tag=f"q_{nm}")
                nc.gpsimd.tensor_tensor(t_[:], s[:], axc(i), mybir.AluOpType.mult)
                qs[nm] = t_
            os_ = {}
            for nm in ["xy", "xz", "yz"]:
                t_ = p_rod.tile([T, NB, JB], F32, tag=f"o_{nm}")
                nc.gpsimd.tensor_tensor(
                    t_[:], u[:], prods[nm][:], mybir.AluOpType.mult
                )
                os_[nm] = t_
            for di, nm in [(0, "xx"), (4, "yy"), (8, "zz")]:
                d_ = p_rod.tile([T, NB, JB], F32, tag=f"d_{nm}")
                nc.vector.tensor_scalar_add(d_[:], prods[nm][:], -1.0)
                nc.vector.tensor_tensor(
                    pf[:, :, :, di], u[:], d_[:], mybir.AluOpType.mult
                )
            for e, o_nm, q_nm, op in [
                (1, "xy", "qz", mybir.AluOpType.subtract),
                (3, "xy", "qz", mybir.AluOpType.add),
                (2, "xz", "qy", mybir.AluOpType.add),
                (6, "xz", "qy", mybir.AluOpType.subtract),
                (5, "yz", "qx", mybir.AluOpType.subtract),
                (7, "yz", "qx", mybir.AluOpType.add),
            ]:
                nc.vector.tensor_tensor(
                    pf[:, :, :, e], os_[o_nm][:], qs[q_nm][:], op
                )

            # ---- per-sample pipeline
            for nb in range(NB):
                # pft8 [103, 2, T]: fp8 DoubleRow K-groups of the delta
                # contraction: group0 = pf rows 0..102, group1 = pf rows
                # 103..188 ++ betas (16) ++ zero pad row.
                pft8 = p_small.tile([KDH, 2, T], F8, tag="pft8")
                nc.vector.memset(pft8[:], 0.0)
                pf_nb = pf[:, nb].rearrange("t j e -> t (j e)")
                tp = ps_sm.tile([KDH, 2, T], BF16, tag="sm")
                nc.tensor.transpose(tp[:, 0, :], pf_nb[:, 0:KDH], ident[:])
                nc.tensor.transpose(
                    tp[0 : PF - KDH, 1, :], pf_nb[:, KDH:PF], ident[:]
                )
                nc.scalar.copy(pft8[:, 0, :], tp[:, 0, :])
                nc.scalar.copy(pft8[0 : PF - KDH, 1, :], tp[0 : PF - KDH, 1, :])
                nc.sync.dma_start(pft8[PF - KDH : PF - KDH + L, 1, :], beta8_d[nb])

                pd8_s = p_big.tile([KDH, 3, 2, N], F8, tag="pd8")
                nc.sync.dma_start(
                    pd8_s[:], pd8_d[nb].rearrange("c k g n -> k c g n")
                )
                wt8_s = p_small.tile([KSH, 2, N], F8, tag="wt8")
                nc.sync.dma_start(wt8_s[:], wt8_d[nb])
                ar8_s = p_small.tile([KSH, 2, 3, 3, T], F8, tag="ar8")
                nc.sync.dma_start(ar8_s[:], ar8_d[nb])
                psit_a = p_small.tile([KT0, N], BF16, tag="psit_a")
                nc.sync.dma_start(psit_a[:], psit_d[nb, 0:KT0])
                psit_b = p_small.tile([KT1, N], BF16, tag="psit_b")
                nc.sync.dma_start(psit_b[:], psit_d[nb, KT0:KT])
                arb_a = p_small.tile([KT0, 3, T], BF16, tag="arb_a")
                nc.sync.dma_start(arb_a[:], arb_d[nb, 0:KT0])
                arb_b = p_small.tile([KT1, 3, T], BF16, tag="arb_b")
                nc.sync.dma_start(arb_b[:], arb_d[nb, KT0:KT])

                # whole-sample output staging: one DMA per sample instead of
                # one per chunk (the SP queue serializes on per-DMA waits)
                outs = p_small.tile([128, NCH, 3, T], BF16, tag="outs")

                for nch in range(NCH):
                    n0 = nch * 128
                    # sm slot: delta [128,3,T] then (same region) S1 [128,3,T]
                    sm = ps_sm.tile([128, 4, T], F32, tag="sm")
                    for c in range(3):
                        nc.tensor.matmul(
                            sm[:, c, :],
                            pd8_s[:, c, :, n0 : n0 + 128],
                            pft8[:],
                            start=True, stop=True, perf_mode=DR,
                        )
                    db = p_mv.tile([128, 3, T], BF16, tag="db")
                    nc.scalar.mul(db[:], sm[:, 0:3, :], 1.0 / PD_SCALE)

                    # ts9 [128, 3i, (3j pad) T] fp8 DoubleRow, one matmul per i
                    ts = ps_ts.tile([128, 3, 4, T], F32, tag="ts")
                    for i in range(3):
                        nc.tensor.matmul(
                            ts[:, i, 0:3, :],
                            wt8_s[:, :, n0 : n0 + 128],
                            ar8_s[:, :, i],
                            start=True, stop=True, perf_mode=DR,
                        )

                    # S1 into the sm slot (region reuse after delta evac)
                    nc.tensor.matmul(
                        sm[:, 0:3, :], psit_a[:, n0 : n0 + 128], arb_a[:],
                        start=True, stop=False,
                    )
                    nc.tensor.matmul(
                        sm[:, 0:3, :], psit_b[:, n0 : n0 + 128], arb_b[:],
                        start=False, stop=True,
                    )

                    dbb = db[:].unsqueeze(1).broadcast_to((128, 3, 3, T))
                    pm = p_mv.tile([128, 3, 3, T], BF16, tag="pm")
                    s1 = p_mv.tile([128, 3, T], BF16, tag="s1")
                    s2 = p_mv.tile([128, 3, T], BF16, tag="s2")
                    if nch % 4 != 3:
                        # ACT evacuates ts9 -> bf16 so the DVE multiply runs
                        # in 2x mode; s1 on DVE.
                        tsb = p_mv.tile([128, 3, 3, T], BF16, tag="tsb")
                        nc.scalar.copy(tsb[:], ts[:, :, 0:3, :])
                        nc.vector.tensor_tensor(
                            pm[:], tsb[:], dbb, mybir.AluOpType.mult
                        )
                        nc.vector.tensor_tensor(
                            s1[:], pm[:, :, 0, :], pm[:, :, 1, :],
                            mybir.AluOpType.add,
                        )
                    else:
                        # DVE multiplies straight from PSUM at 1x; s1 moves
                        # to GPSIMD to unload DVE.
                        nc.vector.tensor_tensor(
                            pm[:], ts[:, :, 0:3, :], dbb, mybir.AluOpType.mult
                        )
                        nc.gpsimd.tensor_tensor(
                            s1[:], pm[:, :, 0, :], pm[:, :, 1, :],
                            mybir.AluOpType.add,
                        )
                    nc.gpsimd.tensor_tensor(
                        s2[:], s1[:], pm[:, :, 2, :], mybir.AluOpType.add
                    )
                    nc.vector.tensor_tensor(
                        outs[:, nch], s2[:], sm[:, 0:3, :], mybir.AluOpType.add
                    )
                nc.sync.dma_start(out_d[nb], outs[:])

    nc.compile()
    return nc


def _prep_core(c, pose_body, trans, betas, A, v_template, shapedirs, posedirs,
               lbs_weights):
    bs = slice(NB * c, NB * (c + 1))
    pose = np.ascontiguousarray(
        pose_body[bs].transpose(1, 0, 2).reshape(T, NB, JB, 3)
    ).astype(np.float32)

    # pd8 [NB, 3, KDH, 2, N]: rows = 189 posedirs + 16 shapedirs + 1 pad,
    # split into the two DoubleRow K-groups, prescaled by PD_SCALE for fp8.
    pdc = posedirs[bs].reshape(NB, PF, N, 3).transpose(0, 3, 1, 2)  # [nb,c,p,n]
    sdc = shapedirs[bs].transpose(0, 2, 3, 1)                       # [nb,c,l,n]
    pcat = np.zeros((NB, 3, KD, N), dtype=np.float32)
    pcat[:, :, 0:PF] = pdc
    pcat[:, :, PF : PF + L] = sdc
    pd8 = np.ascontiguousarray(
        (pcat * PD_SCALE).reshape(NB, 3, 2, KDH, N).transpose(0, 1, 3, 2, 4)
    ).astype(NPF8)

    beta8 = np.ascontiguousarray(betas[bs].transpose(0, 2, 1)).astype(NPF8)

    w = lbs_weights[bs][:, :, 0:J]                                  # [nb, n, k]
    wt8 = np.ascontiguousarray(
        w.transpose(0, 2, 1).reshape(NB, 2, KSH, N).transpose(0, 2, 1, 3)
    ).astype(NPF8)

    # ar8 [NB, KSH, 2, 3i, 3j, T] = A[g*KSH+k, i, j<3, t]
    akij = A[bs, :, :, 0:3, 0:3].transpose(0, 2, 3, 4, 1)           # [nb,k,i,j,t]
    ar8 = np.ascontiguousarray(
        akij.reshape(NB, 2, KSH, 3, 3, T).transpose(0, 2, 1, 3, 4, 5)
    ).astype(NPF8)

    # psit [NB, KT, N]: rows r=k*4+j -> w[n,k]*vth[n,j]; row 208 -> 1
    vth = np.concatenate(
        [v_template[bs], np.ones((NB, N, 1), dtype=np.float32)], axis=2
    )                                                               # [nb, n, 4]
    psi = (w[:, :, :, None] * vth[:, :, None, :]).reshape(NB, N, J * 4)
    psit = np.empty((NB, KT, N), dtype=NPBF16)
    psit[:, 0 : J * 4] = psi.transpose(0, 2, 1).astype(NPBF16)
    psit[:, J * 4] = np.ones((NB, N), dtype=NPBF16)

    # arb [NB, KT, 3, T]: rows r=k*4+j -> A[k,i,j,t]; row 208 -> trans[t,i]
    akji = A[bs, :, :, 0:3, :].transpose(0, 2, 4, 3, 1)             # [nb,k,j,i,t]
    arb = np.empty((NB, KT, 3, T), dtype=NPBF16)
    arb[:, 0 : J * 4] = akji.reshape(NB, J * 4, 3, T).astype(NPBF16)
    arb[:, J * 4] = trans[bs].transpose(0, 2, 1).astype(NPBF16)

    return {
        "pose": pose, "pd8": pd8, "beta8": beta8, "wt8": wt8, "ar8": ar8,
        "psit": psit, "arb": arb,
    }


def kernel(pose_body, trans, betas, A, v_template, shapedirs, posedirs,
           lbs_weights):
    if "nc" not in _CACHED:
        _CACHED["nc"] = _build_nc()
    nc = _CACHED["nc"]

    args = (pose_body, trans, betas, A, v_template, shapedirs, posedirs,
            lbs_weights)
    args = tuple(np.asarray(a, dtype=np.float32) for a in args)
    in_maps = [_prep_core(c, *args) for c in range(NCORES)]

    res = bass_utils.run_bass_kernel_spmd(nc, in_maps, core_ids=list(range(NCORES)))

    # out [NB, 128, NCH, 3, T] per core -> (B, T, N, 3); N = nch*128 + n128
    full = np.stack(
        [res.results[c]["out"].astype(np.float32) for c in range(NCORES)]
    )
    full = full.reshape(B, 128, NCH, 3, T).transpose(0, 4, 2, 1, 3)
    return np.ascontiguousarray(full.reshape(B, T, N, 3).astype(np.float32))
